# revision 26
# baseline (speedup 1.0000x reference)
# Trainium2 Bass kernel for Mistral-style sliding-window GQA attention.
#
# Problem: hidden [2,1024,4096], 32 q-heads / 8 kv-heads, head_dim 128,
# RoPE (neox), causal + sliding-window(512) attention, out proj.
#
# Sharding: tensor-parallel over heads across 8 cores. Core c owns q-heads
# [4c..4c+3] and kv-head c (wq cols 512c:512c+512, wk/wv cols 128c:+128).
# Each core computes its heads' attention output in TRANSPOSED layout
# [feat, tok]; a per-batch AllGather over the 8 cores concatenates the
# feature (partition) axis to give the full [4096, 1024] attn output of that
# batch on every core, and each core then applies its column shard of wo
# ([4096, 512]) to produce out[:, 512c:512c+512]. The host concatenates the
# 8 column shards. The per-batch AG overlaps with the other batch's
# attention compute / out-projection.
#
# All matmuls run in bf16 (fp32 PSUM accumulation); softmax math in fp32.
#
# Layout trick: everything is computed transposed ([feature, token]) so that
# every matmul's contraction operand is already partition-major:
#   QT = wq.T @ hid     via matmul(lhsT=wq_chunk,  rhs=hidT_chunk)
#   KT = wk.T @ hid     via matmul(lhsT=wk_chunk,  rhs=hidT_chunk)
#   VT = wv.T @ hid     via matmul(lhsT=wv_chunk,  rhs=hidT_chunk)
#   V  = VT.T           via 16 PE transposes (V needed k-major for O^T)
#   ST = K_j^T Q        via matmul(lhsT=KT_j,      rhs=QT_span)   [k, q]
#   l  = 1^T A          via matmul(lhsT=ones,      rhs=attnT)     [1, q]
#   OT = V_j^T A        via matmul(lhsT=V_j,       rhs=attnT)     [d, q]
#   out= ag^T @ wo      via matmul(lhsT=ag_chunk,  rhs=wo_chunk)  [tok, oc]
# Softmax over k (partition axis of ST) uses exp with an additive mask (no
# max subtraction -- scores are O(10) here so fp32 exp is safe), a
# ones-matmul for the denominator, and partition_broadcast + reciprocal +
# multiply for the normalization.

import functools

import numpy as np
import ml_dtypes

BF16 = ml_dtypes.bfloat16

B, S, HID = 2, 1024, 4096
T = B * S                     # 2048 flattened tokens
NCORES = 8
D = 128                       # head dim
QH = 4                        # q heads per core
QF = QH * D                   # 512 q features per core
HC = HID // 128               # 32 hidden-dim chunks
NT = 4                        # 512-token chunks
NJ = S // 128                 # 8 k-tiles per batch
WINDOW = 512
NB = WINDOW // 128 + 1        # 5-tile k-span per q-tile
SPAN = NB * 128               # 640
OUTC = HID // NCORES          # 512 out columns per core
SCALE = D ** -0.5


def _build():
    import concourse.mybir as mybir
    import concourse.tile as tile
    from concourse import bacc
    from concourse.masks import make_identity

    f32, bf16 = mybir.dt.float32, mybir.dt.bfloat16
    AF = mybir.ActivationFunctionType
    ALU = mybir.AluOpType

    nc = bacc.Bacc(
        "TRN2", target_bir_lowering=False, debug=False, num_devices=NCORES
    )

    hidT = nc.dram_tensor("hidT", [128, HC * T], bf16, kind="ExternalInput")
    wq = nc.dram_tensor("wq", [128, HC * QF], bf16, kind="ExternalInput")
    wk = nc.dram_tensor("wk", [128, HC * D], bf16, kind="ExternalInput")
    wv = nc.dram_tensor("wv", [128, HC * D], bf16, kind="ExternalInput")
    wo = nc.dram_tensor("wo", [128, HC * OUTC], bf16, kind="ExternalInput")
    cosT = nc.dram_tensor("cosT", [128, T], f32, kind="ExternalInput")
    sinT = nc.dram_tensor("sinT", [128, T], f32, kind="ExternalInput")
    maskT = nc.dram_tensor("maskT", [128, SPAN], bf16, kind="ExternalInput")
    out = nc.dram_tensor("out", [T, OUTC], f32, kind="ExternalOutput")

    with tile.TileContext(nc) as tc:
        with (
            tc.tile_pool(name="ps", bufs=4, space="PSUM") as psp,
            tc.tile_pool(name="consts", bufs=1) as sbp,
            tc.tile_pool(name="hidp", bufs=5) as hidp,
            tc.tile_pool(name="hid2p", bufs=8) as hid2p,
            tc.tile_pool(name="ropep", bufs=2) as ropep,
            tc.tile_pool(name="attnp", bufs=3) as attnp,
            tc.tile_pool(name="miscp", bufs=2) as miscp,
            tc.tile_pool(name="agp", bufs=8) as agp,
            tc.tile_pool(name="dram", bufs=1, space="DRAM") as dramp,
        ):
            def load_split(dst, src, ncols, pieces, skip_first=0, eng=None):
                eng = eng or nc.sync
                step = ncols // pieces
                for i in range(skip_first, pieces):
                    eng.dma_start(
                        dst[:, i * step:(i + 1) * step],
                        src[:, i * step:(i + 1) * step],
                    )

            hidT_r = hidT[:, :].rearrange("p (h t) -> p h t", h=HC)

            def load_hid(n, h2, eng, pool):
                # one DMA covers h-chunks 2*h2 and 2*h2+1 (fewer descriptors,
                # fewer DMA semaphore waits on the PE)
                ht = pool.tile([128, 1024], bf16, tag="hid", name=f"ht{n}_{h2}")
                eng.dma_start(
                    ht[:].rearrange("p (a t) -> p a t", a=2),
                    hidT_r[:, 2 * h2:2 * h2 + 2, n * 512:(n + 1) * 512],
                )
                return ht

            # startup-critical loads first: the h=0..1 chunks of wq/wk/wv
            # feed the very first matmuls; everything else queues behind.
            wq_sb = sbp.tile([128, HC * QF], bf16, name="wq_sb")
            wk_sb = sbp.tile([128, HC * D], bf16, name="wk_sb")
            wv_sb = sbp.tile([128, HC * D], bf16, name="wv_sb")
            nc.sync.dma_start(wq_sb[:, 0:1024], wq[:, 0:1024])
            nc.sync.dma_start(wk_sb[:, 0:1024], wk[:, 0:1024])
            nc.sync.dma_start(wv_sb[:, 0:1024], wv[:, 0:1024])
            pre0 = [load_hid(0, h2, nc.scalar, hidp) for h2 in range(4)]
            load_split(wq_sb, wq, HC * QF, 16, skip_first=1)
            load_split(wk_sb, wk, HC * D, 4, skip_first=1)
            load_split(wv_sb, wv, HC * D, 4, skip_first=1)
            cos_sb = sbp.tile([128, T], f32, name="cos_sb")
            load_split(cos_sb, cosT, T, 4, eng=nc.gpsimd)
            sin_sb = sbp.tile([128, T], f32, name="sin_sb")
            load_split(sin_sb, sinT, T, 4, eng=nc.gpsimd)

            QT_sb = sbp.tile([128, QH * T], bf16, name="QT_sb")
            KT_sb = sbp.tile([128, T], bf16, name="KT_sb")
            VT_sb = sbp.tile([128, T], bf16, name="VT_sb")
            V_sb = sbp.tile([128, T], bf16, name="V_sb")

            # one AllGather per (batch, head-pair): 4 small collectives that
            # hide under the other batch's compute. ag rows come out
            # core-major ([c0 h2p,h2p+1 | c1 h2p,h2p+1 | ...]); the host
            # permutes wo's rows to match (see _prep_in_maps).
            attn_pair = [
                [dramp.tile([256, S], bf16, name=f"attn_local{b}_{p}")
                 for p in range(2)]
                for b in range(B)
            ]
            ag_out = [
                [dramp.tile([256 * NCORES, S], bf16, name=f"ag_out{b}_{p}",
                            addr_space="Shared")
                 for p in range(2)]
                for b in range(B)
            ]

            def rope(stg, stg_sw, dst_sb, col0, tok0):
                # neox rotate-half, 3 full-width DVE ops:
                #   out = x*cos + swap_halves(x)*sin_signed
                # (sin table rows 0:64 arrive pre-negated from the host)
                c = cos_sb[:, tok0:tok0 + 512]
                sg = sin_sb[:, tok0:tok0 + 512]
                t1 = ropep.tile([128, 512], f32, tag="rt1", name="t1")
                t2 = ropep.tile([128, 512], f32, tag="rt2", name="t2")
                nc.vector.tensor_tensor(t1[:], stg[:], c, ALU.mult)
                nc.vector.tensor_tensor(t2[:], stg_sw[:], sg, ALU.mult)
                nc.vector.tensor_tensor(
                    dst_sb[:, col0:col0 + 512], t1[:], t2[:], ALU.add
                )

            def rope_stage(src_ps, tag):
                # psum -> sbuf stages on ACT: a straight copy and a
                # half-swapped copy; the psum slot is released after these
                stg = ropep.tile([128, 512], f32, tag="stg", bufs=2,
                                 name=f"stg{tag}")
                nc.scalar.copy(stg[:], src_ps)
                stg_sw = ropep.tile([128, 512], f32, tag="stgsw", bufs=2,
                                    name=f"stgsw{tag}")
                nc.scalar.copy(stg_sw[0:64, :], src_ps[64:128, :])
                nc.scalar.copy(stg_sw[64:128, :], src_ps[0:64, :])
                return stg, stg_sw

            # ---- phase A: projections, all transposed, weight-stationary ----
            def phase_a(n, eng, pre_tiles=()):
                qa = psp.tile([128, 1024], f32, tag="big", name=f"qa{n}")
                qb = psp.tile([128, 1024], f32, tag="big", name=f"qb{n}")
                kvt = psp.tile([128, 1024], f32, tag="big", name=f"kvt{n}")
                G = 4
                for hg in range(0, HC, G):
                    pairs = []
                    for h2 in range(hg // 2, hg // 2 + G // 2):
                        if h2 < len(pre_tiles):
                            pairs.append(pre_tiles[h2])
                        else:
                            pairs.append(load_hid(n, h2, eng, hidp))
                    hts = [
                        pairs[k // 2][:, (k % 2) * 512:(k % 2) * 512 + 512]
                        for k in range(G)
                    ]
                    # run-length-4 per psum bank: keeps the PE from
                    # micro-idling on per-matmul psum-bank switches
                    for m in range(QH):
                        ps = qa if m < 2 else qb
                        o = (m % 2) * 512
                        for k, h in enumerate(range(hg, hg + G)):
                            nc.tensor.matmul(
                                ps[:, o:o + 512],
                                wq_sb[:, (h * QH + m) * 128:(h * QH + m + 1) * 128],
                                hts[k],
                                start=(h == 0), stop=(h == HC - 1),
                            )
                    for k, h in enumerate(range(hg, hg + G)):
                        nc.tensor.matmul(
                            kvt[:, 0:512],
                            wk_sb[:, h * 128:(h + 1) * 128],
                            hts[k],
                            start=(h == 0), stop=(h == HC - 1),
                        )
                    for k, h in enumerate(range(hg, hg + G)):
                        nc.tensor.matmul(
                            kvt[:, 512:1024],
                            wv_sb[:, h * 128:(h + 1) * 128],
                            hts[k],
                            start=(h == 0), stop=(h == HC - 1),
                        )
                # stage psum -> sbuf on ACT (frees the psum slots quickly),
                # then rope on DVE in fast sbuf-to-sbuf mode; K first so
                # phase B's first score matmuls unblock earliest
                sk, sksw = rope_stage(kvt[:, 0:512], f"K{n}")
                nc.scalar.copy(VT_sb[:, n * 512:(n + 1) * 512], kvt[:, 512:1024])
                rope(sk, sksw, KT_sb, n * 512, n * 512)
                for m in range(QH):
                    ps = qa if m < 2 else qb
                    sq, sqsw = rope_stage(
                        ps[:, (m % 2) * 512:(m % 2) * 512 + 512], f"q{n}_{m}"
                    )
                    rope(sq, sqsw, QT_sb, m * T + n * 512, n * 512)

            # small constants needed by phase B (traced before A so B(b0) can
            # start immediately after A(n0,n1); the DMAs are tiny)
            mask_sb = sbp.tile([128, SPAN], bf16, name="mask_sb")
            nc.sync.dma_start(mask_sb[:], maskT[:, :])
            ones_sb = sbp.tile([128, 1], bf16, name="ones_sb")
            nc.vector.memset(ones_sb[:], 1.0)
            zeros_sb = sbp.tile([128, 512], bf16, name="zeros_sb")
            nc.vector.memset(zeros_sb[:], 0.0)
            ident_sb = sbp.tile([128, 128], bf16, name="ident_sb")
            make_identity(nc, ident_sb[:])
            wo_sb = sbp.tile([128, HC * OUTC], bf16, name="wo_sb")

            # ---- V = VT.T via PE transposes ----
            def v_transpose(tt_lo, tt_hi):
                for tt in range(tt_lo, tt_hi):
                    trp = psp.tile([128, 128], bf16, tag="big", name=f"tr{tt}")
                    nc.tensor.transpose(
                        trp[:], VT_sb[:, tt * 128:(tt + 1) * 128], ident_sb[:]
                    )
                    nc.vector.tensor_copy(V_sb[:, tt * 128:(tt + 1) * 128], trp[:])

            # ---- phase B: windowed attention + per-batch AllGather ----
            def bank_segs(j):
                # q-column range [128j, 128j + w) split at the 512 bank line
                lo, hi = 128 * j, 128 * j + min(NB, NJ - j) * 128
                if lo < 512 < hi:
                    return [(lo, 512), (512, hi)]
                return [(lo, hi)]

            def phase_b(b):
                for m in range(QH):
                    l_ps = psp.tile([128, 1024], f32, tag="big", name=f"l{b}_{m}")
                    o_ps = psp.tile([128, 1024], f32, tag="big", name=f"o{b}_{m}")

                    def norm_half(hb, mm=m, bb=b, l_ps=l_ps, o_ps=o_ps):
                        # oT[:, c0:c0+512] = o_ps/l via bcast(1/l); runs as
                        # soon as that psum bank's accumulation stops
                        c0 = hb * 512
                        l_sb = miscp.tile([1, 512], f32, tag="lsb",
                                          name=f"l_sb{bb}{mm}{hb}")
                        nc.scalar.copy(l_sb[:], l_ps[0:1, c0:c0 + 512])
                        bc = miscp.tile([128, 512], f32, tag="bcast",
                                        name=f"bc{bb}{mm}{hb}")
                        nc.gpsimd.partition_broadcast(bc[:], l_sb[:])
                        bcr = miscp.tile([128, 512], f32, tag="bcr",
                                         name=f"bcr{bb}{mm}{hb}")
                        nc.vector.reciprocal_approx_fast(bcr[:], bc[:])
                        oT = miscp.tile([128, 512], bf16, tag="osb",
                                        name=f"oT{bb}{mm}{hb}")
                        nc.vector.tensor_tensor(
                            oT[:], o_ps[:, c0:c0 + 512], bcr[:], ALU.mult
                        )
                        nc.gpsimd.dma_start(
                            attn_pair[bb][mm // 2][
                                (mm % 2) * 128:(mm % 2) * 128 + 128,
                                c0:c0 + 512], oT[:]
                        )

                    at_tiles = {}

                    def scores(j):
                        # scores^T tile [k, q-span] + PE-side mask add + exp
                        w = min(NB, NJ - j) * 128
                        sc = psp.tile(
                            [128, 1024], f32, tag="big", name=f"sc{b}_{m}_{j}"
                        )
                        kslice = KT_sb[:, b * S + j * 128:b * S + (j + 1) * 128]
                        q0 = m * T + b * S + j * 128
                        for o in range(0, w, 512):
                            nw = min(512, w - o)
                            nc.tensor.matmul(
                                sc[:, o:o + nw], kslice,
                                QT_sb[:, q0 + o:q0 + o + nw],
                                start=True, stop=False,
                            )
                            nc.tensor.matmul(
                                sc[:, o:o + nw], ident_sb[:],
                                mask_sb[:, o:o + nw],
                                start=False, stop=True,
                            )
                        at = attnp.tile(
                            [128, SPAN], bf16, tag="attn", bufs=3,
                            name=f"at{b}{m}{j}"
                        )
                        nc.scalar.activation(at[:, :w], sc[:, :w], AF.Exp)
                        at_tiles[j] = at

                    def acc(j, which):
                        # span accumulation per psum bank (l: ones, oT: V_j)
                        at = at_tiles[j]
                        vslice = V_sb[:, (b * NJ + j) * 128:(b * NJ + j + 1) * 128]
                        for (lo, hi) in bank_segs(j):
                            sp = (j == 3) if hi <= 512 else (j == NJ - 1)
                            st = (j == 0 and hi <= 512)
                            a = at[:, lo - 128 * j: hi - 128 * j]
                            if which == "l":
                                nc.tensor.matmul(
                                    l_ps[0:1, lo:hi], ones_sb[:], a,
                                    start=st, stop=sp,
                                )
                            else:
                                nc.tensor.matmul(
                                    o_ps[:, lo:hi], vslice, a,
                                    start=st, stop=sp,
                                )

                    # trace order keeps the PE fed while the previous head's
                    # normalize chain releases this head's o_ps slot: two
                    # score tiles and the l accumulation come first; the
                    # bank-B zero-init dummies go right before first use
                    scores(0)
                    scores(1)
                    nc.tensor.matmul(
                        l_ps[0:1, 512:1024], ones_sb[:], zeros_sb[:],
                        start=True, stop=False,
                    )
                    acc(0, "l")
                    acc(1, "l")
                    nc.tensor.matmul(
                        o_ps[:, 512:1024], zeros_sb[:, 0:128], zeros_sb[:],
                        start=True, stop=False,
                    )
                    acc(0, "o")
                    acc(1, "o")
                    for j in range(2, NJ):
                        scores(j)
                        acc(j, "l")
                        acc(j, "o")
                        if j == 3:
                            norm_half(0)
                    norm_half(1)
                    if m % 2 == 1:
                        # head-pair AllGather as soon as both heads are out
                        nc.gpsimd.collective_compute(
                            "AllGather",
                            ALU.bypass,
                            ins=[attn_pair[b][m // 2][:, :]],
                            outs=[ag_out[b][m // 2][:, :]],
                            replica_groups=[list(range(NCORES))],
                        )

            # ---- phase D: out projection on this core's wo column shard ----
            def phase_d(b):
                for pp in range(S // 512):
                    ops = [
                        psp.tile([128, 1024], f32, tag="big",
                                 name=f"op{b}_{pp}_{q}")
                        for q in range(2)
                    ]
                    for afg in range(0, HC, 4):
                        ag_ts = []
                        for af in range(afg, afg + 4):
                            ag_t = agp.tile(
                                [128, 512], bf16, tag="ag", bufs=10,
                                name=f"ag{b}_{pp}_{af}"
                            )
                            src = ag_out[b][af // 16]
                            a16 = af % 16
                            eng2 = nc.sync if af % 2 == 0 else nc.scalar
                            eng2.dma_start(
                                ag_t[:],
                                src[a16 * 128:(a16 + 1) * 128,
                                    pp * 512:(pp + 1) * 512],
                            )
                            ag_ts.append(ag_t)
                        # run-length-4 per psum bank
                        for tt in range(4):
                            for k, af in enumerate(range(afg, afg + 4)):
                                nc.tensor.matmul(
                                    ops[tt // 2][:, (tt % 2) * 512:(tt % 2) * 512 + 512],
                                    ag_ts[k][:, tt * 128:(tt + 1) * 128],
                                    wo_sb[:, af * OUTC:(af + 1) * OUTC],
                                    start=(af == 0), stop=(af == HC - 1),
                                )
                    for q in range(2):
                        ob = miscp.tile([128, 1024], f32, tag="ob",
                                        name=f"ob{b}_{pp}_{q}")
                        nc.vector.tensor_copy(ob[:], ops[q][:])
                        r0 = b * S + pp * 512 + q * 256
                        nc.sync.dma_start(out[r0:r0 + 128, :], ob[:, 0:512])
                        nc.sync.dma_start(out[r0 + 128:r0 + 256, :], ob[:, 512:1024])

            # ---- orchestration: batch-interleaved so every AllGather hides
            # under the other batch's projections / attention / out-proj ----
            phase_a(0, nc.scalar, pre0)
            phase_a(1, nc.scalar)
            v_transpose(0, NJ)
            # prefetch n=2's first half on the (now idle) sync queue so
            # phase A(2) can start the moment B(b0)'s matmuls finish
            pre2 = [load_hid(2, h2, nc.sync, hid2p) for h2 in range(8)]
            phase_b(0)              # ... AG(b0,p0), AG(b0,p1) in flight
            load_split(wo_sb, wo, HC * OUTC, 16, eng=nc.scalar)
            phase_a(2, nc.sync, pre2)
            phase_a(3, nc.sync)
            v_transpose(NJ, 2 * NJ)
            phase_b(1)              # ... AG(b1,p0), AG(b1,p1) in flight
            phase_d(0)
            phase_d(1)

    nc.compile()
    return nc


@functools.lru_cache(maxsize=1)
def _get_nc():
    return _build()


def _prep_in_maps(hidden_states, wq, wk, wv, wo, cos, sin):
    hs = np.ascontiguousarray(np.asarray(hidden_states, np.float32)).reshape(T, HID)
    hidT = hs.T.reshape(HC, 128, T).transpose(1, 0, 2).reshape(128, HC * T)
    hidT = np.ascontiguousarray(hidT).astype(BF16)

    wq = np.asarray(wq, np.float32) * SCALE
    wk = np.asarray(wk, np.float32)
    wv = np.asarray(wv, np.float32)
    wo = np.asarray(wo, np.float32)

    cosT = np.ascontiguousarray(np.asarray(cos, np.float32).T)  # [64, S]
    sinT = np.ascontiguousarray(np.asarray(sin, np.float32).T)
    cosT2 = np.concatenate([cosT, cosT], axis=1)   # [64, T]
    sinT2 = np.concatenate([sinT, sinT], axis=1)
    cos128 = np.concatenate([cosT2, cosT2], axis=0)  # [128, T]
    sin128 = np.concatenate([-sinT2, sinT2], axis=0)

    r = np.arange(128)[:, None]
    c = np.arange(128)[None, :]
    SL = np.where(c < r, -1e30, 0.0)  # diag tile: invalid where q < k
    SU = np.where(c > r, -1e30, 0.0)  # window-edge tile: invalid where q-k > W
    maskadd = np.concatenate(
        [SL, np.zeros((128, SPAN - 256)), SU], axis=1
    ).astype(BF16)

    def shard_w(w, cols, core):
        ws = w[:, core * cols:(core + 1) * cols]
        return np.ascontiguousarray(
            ws.reshape(HC, 128, cols).transpose(1, 0, 2).reshape(128, HC * cols)
        ).astype(BF16)

    def shard_wo(w, core):
        # chunk ci of phase D reads rows a16=ci%16 of head-pair gather
        # p=ci//16, whose rows are (core c2=a16//2, head-in-pair hp=a16%2)
        # -> global head 4*c2 + 2*p + hp
        ws = w[:, core * OUTC:(core + 1) * OUTC]
        blocks = []
        for ci in range(HC):
            p, a16 = divmod(ci, 16)
            c2, hp = divmod(a16, 2)
            g = 4 * c2 + 2 * p + hp
            blocks.append(ws[g * 128:(g + 1) * 128, :])
        arr = np.stack(blocks, 0)
        return np.ascontiguousarray(
            arr.transpose(1, 0, 2).reshape(128, HC * OUTC)
        ).astype(BF16)

    in_maps = []
    for cidx in range(NCORES):
        in_maps.append({
            "hidT": hidT,
            "wq": shard_w(wq, QF, cidx),
            "wk": shard_w(wk, D, cidx),
            "wv": shard_w(wv, D, cidx),
            "wo": shard_wo(wo, cidx),
            "cosT": cos128,
            "sinT": sin128,
            "maskT": maskadd,
        })
    return in_maps


def run(inputs, trace=False, **spmd_kwargs):
    from concourse.bass_utils import run_bass_kernel_spmd

    window = int(np.asarray(inputs["window"]))
    assert window == WINDOW, f"kernel compiled for window={WINDOW}, got {window}"
    nc = _get_nc()
    in_maps = _prep_in_maps(
        inputs["hidden_states"], inputs["wq"], inputs["wk"], inputs["wv"],
        inputs["wo"], inputs["cos"], inputs["sin"],
    )
    res = run_bass_kernel_spmd(
        nc, in_maps, list(range(NCORES)), trace=trace, **spmd_kwargs
    )
    parts = [np.asarray(res.results[i]["out"], np.float32) for i in range(NCORES)]
    full = np.concatenate(parts, axis=1).reshape(B, S, HID)
    return full, res


def kernel(**inputs):
    return run(inputs, trace=False)[0]
